# revision 1
# baseline (speedup 1.0000x reference)
"""Trainium2 Bass kernel for the CoincidenceDetector problem.

Math (reference):
    pt = 20 - 15*sigmoid(patterns)                     # (N, D)
    dt = qt[b,d] - pt[n,d]
    kappa = where(|dt| < 5, exp(-|dt|/3), 0)
    S[b,n] = sum_d kappa * |w[d]|

Device formulation (per core, patterns' N sharded 8 ways, n on partitions):
    s  = sigmoid(patterns)                   (fp16; |dt| = 15*|s - q''|)
    q'' = (20 - qt)/15                       (host-precomputed, replicated
                                              across partitions, fp16)
    per 128-pattern tile (free dim = 64 batches x 256 features = 16384):
      d1    = s (broadcast over b) - q''               DVE tensor_tensor
      a'    = d1 & 0x7fff   (abs via sign-bit clear)   DVE tensor_scalar
      E     = exp(-5*a')                               ScalarE activation
      M     = a' < 1/3      (coincidence window)       DVE tensor_scalar
      kappa = E * M                                    DVE tensor_tensor
      S     = reduce_add(kappa, over d)                DVE tensor_reduce
    -> (128 patterns, 64 batches) fp32 scores per tile.

The execution environment prices instructions (~30-60us each) far above
their architectural cost, so the kernel is shaped to minimize instruction
count: ~100 instructions/core instead of a matmul-based formulation.
"""

import numpy as np

import concourse.bass as bass
import concourse.mybir as mybir
import concourse.tile as tile
from concourse.bass_utils import run_bass_kernel_spmd

F32 = mybir.dt.float32
F16 = mybir.dt.float16
U16 = mybir.dt.uint16
AF = mybir.ActivationFunctionType
ALU = mybir.AluOpType

B, N, D = 64, 16384, 256
N_SPLIT = 8
N_CORES = 8
P = 128

ONE_THIRD = float(np.float32(1.0) / np.float32(3.0))

_PROGRAM_CACHE = {}


def _split_multi_waits(nc, max_inline=1):
    """The walrus codegen in this container supports only a small number of
    embedded sync-wait commands per instruction (1 for most engine ops).
    Tile's wait assignment can attach several.  Hoist all but `max_inline`
    waits of every instruction onto standalone EventSemaphore instructions
    (one wait each) inserted immediately before it on the same engine —
    semantically identical, the engine queue stalls the same way."""
    for bbname, bass_bb in list(nc.bb_map.items()):
        insts = bass_bb.bb.instructions
        i = 0
        while i < len(insts):
            inst = insts[i]
            si = inst.sync_info
            if si is not None and si.on_wait and len(si.on_wait) > max_inline:
                waits = list(si.on_wait)
                keep = waits[-max_inline:] if max_inline else []
                hoist = waits[: len(waits) - max_inline]
                carriers = []
                for w in hoist:
                    ev = mybir.InstEventSemaphore(
                        name=nc.get_next_instruction_name(),
                        engine=inst.engine,
                        ins=[],
                        outs=[],
                        sync_info=mybir.SyncInfo(on_wait=[w], on_update=[]),
                    )
                    nc.register_instruction(ev)
                    carriers.append(ev)
                inst.sync_info = mybir.SyncInfo(
                    on_wait=keep, on_update=list(si.on_update)
                )
                insts[i:i] = carriers
                i += len(carriers)
            i += 1


def build_program(n_loc=N // N_SPLIT, b_loc=B, repeat=1, with_weights=False,
                  merge=1):
    """Build the single-core Bass program (same program runs SPMD on all
    cores; per-core data differs only in the patterns shard).

    repeat > 1 re-runs the whole compute that many times — used only for
    wall-clock differential timing.  merge processes that many 128-pattern
    tiles per instruction (bigger free dims, fewer instructions)."""
    if with_weights:
        merge = 1  # weighted path keeps the simpler 3D APs
    ntile = n_loc // P           # 16
    qd = b_loc * D               # 16384
    fd = merge * qd
    nc = bass.Bass("TRN2")

    patn = nc.dram_tensor("patn", [P, ntile * D], F32, kind="ExternalInput")
    qrep = nc.dram_tensor("qrep", [P, qd], F16, kind="ExternalInput")
    wrep = None
    if with_weights:
        wrep = nc.dram_tensor("wrep", [P, D], F16, kind="ExternalInput")
    out = nc.dram_tensor("out", [P, ntile * b_loc], F32, kind="ExternalOutput")

    with tile.TileContext(nc) as tc:
        with (
            tc.tile_pool(name="const", bufs=1) as constp,
            tc.tile_pool(name="work", bufs=1) as wp,
        ):
            pat_sb = constp.tile([P, ntile * D], F32, tag="pat", name="pat")
            nc.sync.dma_start(pat_sb[:], patn[:])
            q_sb = constp.tile([P, qd], F16, tag="q", name="q")
            nc.sync.dma_start(q_sb[:], qrep[:])
            w_sb = None
            if with_weights:
                w_sb = constp.tile([P, D], F16, tag="w", name="w")
                nc.sync.dma_start(w_sb[:], wrep[:])
            s_sb = constp.tile([P, ntile * D], F16, tag="s", name="s")
            s_out = constp.tile([P, ntile * b_loc], F32, tag="so", name="so")

            q_3d = q_sb[:].rearrange("p (b d) -> p b d", b=b_loc)

            for _rep in range(repeat):
                nc.scalar.activation(s_sb[:], pat_sb[:], AF.Sigmoid)
                for t in range(0, ntile, merge):
                    if merge == 1:
                        s_bc = (
                            s_sb[:, t * D:(t + 1) * D]
                            .rearrange("p (u d) -> p u d", u=1)
                            .broadcast_to([P, b_loc, D])
                        )
                        q_in = q_3d
                    else:
                        s_bc = (
                            s_sb[:, t * D:(t + merge) * D]
                            .rearrange("p (t u d) -> p t u d", t=merge, u=1)
                            .broadcast_to([P, merge, b_loc, D])
                        )
                        q_in = (
                            q_sb[:]
                            .rearrange("p (u b d) -> p u b d", u=1, b=b_loc)
                            .broadcast_to([P, merge, b_loc, D])
                        )
                    d1 = wp.tile([P, fd], F16, tag="d1", name="d1")
                    if merge == 1:
                        d1_3d = d1[:].rearrange("p (b d) -> p b d", b=b_loc)
                    else:
                        d1_3d = d1[:].rearrange(
                            "p (t b d) -> p t b d", t=merge, b=b_loc
                        )
                    nc.vector.tensor_tensor(
                        out=d1_3d, in0=s_bc, in1=q_in, op=ALU.subtract
                    )
                    # abs in place: clear the fp16 sign bit
                    nc.vector.tensor_scalar(
                        d1.bitcast(U16)[:], d1.bitcast(U16)[:],
                        0x7FFF, None, ALU.bitwise_and,
                    )
                    e_t = wp.tile([P, fd], F16, tag="e", name="e")
                    nc.scalar.activation(e_t[:], d1[:], AF.Exp, scale=-5.0)
                    # kappa = (a' < 1/3) * E fused in one pass
                    if merge == 1:
                        m_t = wp.tile([P, fd], F16, tag="m", name="m")
                        kap = m_t
                    else:
                        kap = e_t  # in place over E to stay within SBUF
                    nc.vector.scalar_tensor_tensor(
                        out=kap[:], in0=d1[:], scalar=ONE_THIRD, in1=e_t[:],
                        op0=ALU.is_lt, op1=ALU.mult,
                    )
                    if merge == 1:
                        m_3d = kap[:].rearrange("p (b d) -> p b d", b=b_loc)
                    else:
                        m_3d = kap[:].rearrange(
                            "p (t b d) -> p t b d", t=merge, b=b_loc
                        )
                    if with_weights:
                        w_bc = (
                            w_sb[:]
                            .rearrange("p (u d) -> p u d", u=1)
                            .broadcast_to([P, b_loc, D])
                        )
                        nc.vector.tensor_tensor(
                            out=m_3d, in0=m_3d, in1=w_bc, op=ALU.mult
                        )
                    nc.vector.tensor_reduce(
                        out=s_out[:, t * b_loc:(t + merge) * b_loc],
                        in_=m_3d,
                        axis=mybir.AxisListType.X,
                        op=ALU.add,
                    )

            nc.sync.dma_start(out[:], s_out[:])

    _split_multi_waits(nc)
    return nc


def _get_program(repeat=1, with_weights=False):
    key = (repeat, with_weights)
    if key not in _PROGRAM_CACHE:
        _PROGRAM_CACHE[key] = build_program(
            repeat=repeat, with_weights=with_weights, merge=2
        )
    return _PROGRAM_CACHE[key]


def make_in_maps(query_times, patterns, weights, n_loc, b_loc,
                 with_weights=False):
    """Host-side input marshalling: layout transforms plus the tiny
    O(B*D + D) affine precomputation."""
    qt = np.asarray(query_times, dtype=np.float32)
    pat = np.asarray(patterns, dtype=np.float32)
    w = np.asarray(weights, dtype=np.float32)
    ntile = n_loc // P

    q2 = ((np.float32(20.0) - qt) / np.float32(15.0)).astype(np.float16)
    qrep = np.ascontiguousarray(
        np.broadcast_to(q2.reshape(1, b_loc * D), (P, b_loc * D))
    )
    shared = {"qrep": qrep}
    if with_weights:
        w16 = np.abs(w).astype(np.float16)
        shared["wrep"] = np.ascontiguousarray(
            np.broadcast_to(w16.reshape(1, D), (P, D))
        )

    in_maps = []
    for c in range(N_CORES):
        shard = pat[c * n_loc:(c + 1) * n_loc]  # (n_loc, D)
        patn = np.ascontiguousarray(
            shard.reshape(ntile, P, D).transpose(1, 0, 2).reshape(P, ntile * D)
        )
        in_maps.append({"patn": patn, **shared})
    return in_maps


def kernel(query_times, patterns, weights, _trace=False, _repeat=1):
    n_loc = N // N_SPLIT
    b_loc = B
    ntile = n_loc // P

    w = np.asarray(weights, dtype=np.float32)
    with_weights = not np.all(np.abs(w) == 1.0)

    nc = _get_program(repeat=_repeat, with_weights=with_weights)
    in_maps = make_in_maps(query_times, patterns, weights, n_loc, b_loc,
                           with_weights=with_weights)

    res = run_bass_kernel_spmd(nc, in_maps, list(range(N_CORES)), trace=_trace)

    S = np.empty((B, N), np.float32)
    for c in range(N_CORES):
        o = res.results[c]["out"]  # (P, ntile*b_loc)
        o = o.reshape(P, ntile, b_loc).transpose(2, 1, 0).reshape(B, n_loc)
        S[:, c * n_loc:(c + 1) * n_loc] = o
    if _trace:
        return S, res
    return S

